# revision 54
# baseline (speedup 1.0000x reference)
"""Trainium2 Bass kernel for nn_MultiHeadAttention_37623913513495.

Multi-head attention with rotary embeddings and a relative-position bias
(einsum('bhid,ijd->bhij', q, rel_pos[j-i+T-1])), sharded over 8 NeuronCores
as 4 batches x 2 head-groups (8 heads each). Host sums the two partial
outputs per batch and adds the bias.

Device-side structure (per core), v2 engine-balanced pipeline:
  - host ships x already transposed (xT [C, T]); q/k projections produce
    qT/kT in [d, T] layout, v in [T, d]; rotary on DVE; 1/sqrt(hs) folded
    into wq on the host, E shipped pre-multiplied by 8 to compensate
  - the relative-position "skew" rel[i,j] = A[i, j-i+c] is realized by
    writing RAW A windows to DRAM (f16, evacuated from PSUM by the gpsimd
    engine) and re-reading them with a strided diagonal access pattern
  - scores: S psum + rel added on DVE -> f16; single Exp on ACT with
    fused row-sums via accum_out
  - P is transposed by PE is_transpose matmuls (f16 PSUM out, cheap evac)
  - AV is computed in [i, d] orientation (N=64 matmuls); per-head
    normalization by 1/rowsum is a per-partition scalar multiply on the
    AV output; the 8-head attn block is re-transposed (4 PE transposes)
    into attnT for the output projection, which is pipelined per block
"""

import numpy as np

HS = 64           # head size
NH = 16           # total heads
SEQ = 1024        # sequence length
EMB = 1024        # embedding dim
BATCH = 4
N_CORES = 8
HC = NH // 2      # heads per core

_cache = {}


def _build_nc(T, C, D, reps=1):
    import concourse.bass as bass
    import concourse.bacc as bacc
    import concourse.mybir as mybir
    import concourse.tile as tile
    from concourse.masks import make_identity

    dt = mybir.dt
    f32, f16 = dt.float32, dt.float16
    AF = mybir.ActivationFunctionType
    ALU = mybir.AluOpType

    P = 128
    NB = T // P              # row blocks
    KC = C // P              # contraction chunks over C
    DT = D // P              # qT/kT partition tiles
    HPT = P // HS            # heads per qT tile (2)
    HCL = D // HS            # heads on this core
    JH = min(512, T)         # half width
    NJH = T // JH            # halves per row
    WF = T + P               # A-window width
    NCH = T // P             # j chunks for PT/AV
    ACH = (512, 512, WF - 1024)   # A psum chunk widths (bank-aligned)

    nc = bacc.Bacc(None, target_bir_lowering=False, debug=False)

    xT_d = nc.dram_tensor("xT", [C, T], f16, kind="ExternalInput")
    wq_d = nc.dram_tensor("wq", [C, D], f16, kind="ExternalInput")
    wk_d = nc.dram_tensor("wk", [C, D], f16, kind="ExternalInput")
    wv_d = nc.dram_tensor("wv", [C, D], f16, kind="ExternalInput")
    wo_d = nc.dram_tensor("wo", [D, C], f16, kind="ExternalInput")
    cos_d = nc.dram_tensor("cosT", [P, T], f16, kind="ExternalInput")
    sin_d = nc.dram_tensor("sinS", [P, T], f16, kind="ExternalInput")
    et_d = nc.dram_tensor("et8", [P, 2 * T], f16, kind="ExternalInput")
    y_d = nc.dram_tensor("y", [T, C], f16, kind="ExternalOutput")

    with tile.TileContext(nc) as tc:
        with (
            tc.tile_pool(name="const", bufs=1) as const,
            tc.tile_pool(name="persist", bufs=1) as persist,
            tc.tile_pool(name="asb", bufs=8) as asb_pool,
            tc.tile_pool(name="relsb", bufs=26) as rel_pool,
            tc.tile_pool(name="sssb", bufs=3) as ss_pool,
            tc.tile_pool(name="psb", bufs=11) as p_pool,
            tc.tile_pool(name="ptsb", bufs=3) as pt_pool,
            tc.tile_pool(name="attnsb", bufs=2) as attn_pool,
            tc.tile_pool(name="small", bufs=2) as small,
            tc.tile_pool(name="outsb", bufs=4) as out_pool,
            tc.tile_pool(name="psum", bufs=2, space="PSUM") as psx,
            tc.tile_pool(name="adram", bufs=32, space="DRAM") as adram,
        ):
            for _rep in range(reps):
                # ---------------- xT load (sync ring) ----------------
                xT_sb, xT_free = [], []
                for cb in range(KC):
                    t, fr = tc.tile([P, T], f16, name=f"xT_{cb}")
                    nc.sync.dma_start(out=t, in_=xT_d[cb * P:(cb + 1) * P, :])
                    xT_sb.append(t)
                    xT_free.append(fr)

                # ---------------- constants ----------------
                ident_b = const.tile([P, P], f16)
                make_identity(nc, ident_b)
                cos_sb = const.tile([P, T], f16)
                sin_sb = const.tile([P, T], f16)
                et_sb = const.tile([P, 2 * T], f16)

                def load_w(wd, name, eng):
                    tiles, frees = [], []
                    for kb in range(KC):
                        t, fr = tc.tile([P, D], f16, name=f"{name}_{kb}")
                        eng.dma_start(out=t, in_=wd[kb * P:(kb + 1) * P, :])
                        tiles.append(t)
                        frees.append(fr)
                    return tiles, frees

                # rotary pairs (d, d+32) sit 16 apart within a 32-partition
                # quadrant (host permutes weights/E/tables to match).
                shuf_mask = [(i + 16) % 32 for i in range(32)]

                qT_sb = [persist.tile([P, T], f16, name=f"qT_{mb}", tag=f"qT{mb}")
                         for mb in range(DT)]
                kT_sb = [persist.tile([P, T], f16, name=f"kT_{mb}", tag=f"kT{mb}")
                         for mb in range(DT)]
                v_sb = [persist.tile([P, D], f16, name=f"v_{tb}", tag=f"v{tb}")
                        for tb in range(NB)]
                attnT_sb = persist.tile([P, DT, T], f16, name="attnT", tag="attnT")

                rel_tiles = [dict() for _ in range(HCL)]
                p_tiles = [dict() for _ in range(HCL)]
                pt_tiles = [dict() for _ in range(HCL)]
                sums_blk = {}
                av_blk = {}
                attn_blk = {}

                def mode_add(k):
                    # every 4th stream step computes exp(S+rel) via a DVE add
                    # of raw rel instead of the exp(S)*exp(A) product -- this
                    # shifts work from the exp-bound ACT engine onto DVE.
                    return k % 8 == 5 and k >= HCL

                def emit_A(h, ib):
                    """A window (head h, block ib) -> DRAM -> skew readback."""
                    par = (h % HPT) * HS
                    qtile = qT_sb[h // HPT]
                    i0 = ib * P
                    w0 = (T - P) - i0
                    a_ps = psx.tile([P, WF], f32, name="a_ps", tag="a",
                                    bufs=1)
                    off = 0
                    for cw in ACH:
                        nc.tensor.matmul(
                            a_ps[:, off:off + cw],
                            qtile[par:par + HS, i0:i0 + P],
                            et_sb[par:par + HS, w0 + off:w0 + off + cw],
                            start=True, stop=True)
                        off += cw
                    a_sb = asb_pool.tile([P, WF], f16, name="a_sb", tag="a_sb")
                    if mode_add(ib * HCL + h):
                        nc.vector.tensor_copy(a_sb, a_ps)
                    else:
                        nc.scalar.activation(a_sb, a_ps, AF.Exp)
                    a_dr = adram.tile([P, WF], f16, name="a_dr", tag="a_dr")
                    nc.sync.dma_start(out=a_dr, in_=a_sb)
                    rel = rel_pool.tile([P, T], f16, name="rel", tag="rel")
                    skew = bass.AP(
                        tensor=a_dr.tensor,
                        offset=a_dr.offset + (P - 1),
                        ap=[[WF - 1, P], [1, T]],
                    )
                    nc.sync.dma_start(out=rel, in_=skew)
                    rel_tiles[h][ib] = rel

                def emit_SE(h, ib):
                    """scores -> exp(S+rel) with fused rowsums (two forms)."""
                    par = (h % HPT) * HS
                    qtile, ktile = qT_sb[h // HPT], kT_sb[h // HPT]
                    i0 = ib * P
                    rel = rel_tiles[h].pop(ib)
                    s_ps = psx.tile([P, T], f32, name="s_ps", tag="s", bufs=1)
                    for jh in range(NJH):
                        sl = slice(jh * JH, (jh + 1) * JH)
                        nc.tensor.matmul(
                            s_ps[:, sl],
                            qtile[par:par + HS, i0:i0 + P],
                            ktile[par:par + HS, sl],
                            start=True, stop=True)
                    if ib not in sums_blk:
                        sums_blk[ib] = small.tile([P, HCL], f32, name="sums",
                                                  tag="sums", bufs=2)
                    p_sb = p_pool.tile([P, T], f16, name="p_sb", tag="p_sb")
                    if mode_add(ib * HCL + h):
                        ss_sb = ss_pool.tile([P, T], f16, name="ss_sb",
                                             tag="ss")
                        nc.vector.tensor_add(ss_sb, s_ps, rel)
                        nc.scalar.activation(
                            p_sb, ss_sb, AF.Exp,
                            accum_out=sums_blk[ib][:, h:h + 1])
                    else:
                        es_sb = ss_pool.tile([P, T], f16, name="es_sb",
                                             tag="ss")
                        nc.scalar.activation(es_sb, s_ps, AF.Exp)
                        nc.vector.scalar_tensor_tensor(
                            p_sb, es_sb, 1.0, rel,
                            ALU.mult, ALU.mult,
                            accum_out=sums_blk[ib][:, h:h + 1])
                    p_tiles[h][ib] = p_sb

                def emit_PT(h, ib):
                    """transpose P via PE is_transpose (f16 psum out)."""
                    p_sb = p_tiles[h].pop(ib)
                    ptp = psx.tile([P, NCH * P], f16, name="ptp", tag="pt",
                                   bufs=2)
                    for jc in range(NCH):
                        nc.tensor.transpose(
                            ptp[:, jc * P:(jc + 1) * P],
                            p_sb[:, jc * P:(jc + 1) * P], ident_b)
                    pt_sb = pt_pool.tile([P, NCH, P], f16, name="pt_sb",
                                         tag="pt_sb")
                    nc.vector.tensor_copy(
                        pt_sb.rearrange("p a b -> p (a b)"), ptp)
                    pt_tiles[h][ib] = pt_sb

                def emit_AV(h, ib):
                    """AV in [i, d] orientation into the shared block psum."""
                    pt_sb = pt_tiles[h].pop(ib)
                    if ib not in av_blk:
                        av_blk[ib] = psx.tile([P, D], f32, name="av_ps",
                                              tag="av", bufs=1)
                    av = av_blk[ib]
                    for jc in range(NCH):
                        nc.tensor.matmul(
                            av[:, h * HS:(h + 1) * HS],
                            pt_sb[:, jc, :],
                            v_sb[jc][:, h * HS:(h + 1) * HS],
                            start=(jc == 0), stop=(jc == NCH - 1))

                def emit_norm(ib):
                    """normalize the 8-head attn block by 1/rowsum."""
                    sums = sums_blk.pop(ib)
                    av = av_blk.pop(ib)
                    rec = small.tile([P, HCL], f32, name="rec", tag="rec",
                                     bufs=2)
                    nc.vector.reciprocal(rec, sums)
                    attn_sb = attn_pool.tile([P, D], f16, name="attn_sb",
                                             tag="attn")
                    for h in range(HCL):
                        nc.vector.tensor_scalar_mul(
                            attn_sb[:, h * HS:(h + 1) * HS],
                            av[:, h * HS:(h + 1) * HS],
                            rec[:, h:h + 1])
                    attn_blk[ib] = attn_sb

                def emit_tr(ib):
                    """re-transpose the attn block into attnT."""
                    attn_sb = attn_blk.pop(ib)
                    trp = psx.tile([P, DT * P], f16, name="trp", tag="pt",
                                   bufs=2)
                    for k in range(DT):
                        nc.tensor.transpose(
                            trp[:, k * P:(k + 1) * P],
                            attn_sb[:, k * P:(k + 1) * P], ident_b)
                    nc.scalar.copy(
                        attnT_sb[:, :, ib * P:(ib + 1) * P],
                        trp.rearrange("p (a b) -> p a b", a=DT))

                def emit_out(ib, ch):
                    """final projection for row-block ib (all heads done)."""
                    op = psx.tile([P, JH], f32, name="ops", tag="pt",
                                  bufs=2)
                    for hc in range(DT):
                        nc.tensor.matmul(
                            op,
                            attnT_sb[:, hc, ib * P:(ib + 1) * P],
                            wo_sb[hc][:, ch * JH:(ch + 1) * JH],
                            start=(hc == 0), stop=(hc == DT - 1))
                    o_sb = out_pool.tile([P, JH], f16, name="o_sb",
                                         tag="o_sb")
                    nc.vector.tensor_copy(o_sb, op)
                    nc.scalar.dma_start(
                        out=y_d[ib * P:(ib + 1) * P, ch * JH:(ch + 1) * JH],
                        in_=o_sb)

                # ---------------- q/k projections + rotary ----------------
                rot_tmp = p_pool.tile([P, T], f16, name="rot_tmp", tag="rot",
                                      bufs=1)
                first_w = True
                for (wd, wname, dest) in ((wq_d, "wq", qT_sb), (wk_d, "wk", kT_sb)):
                    w_sb, w_frees = load_w(
                        wd, wname, nc.scalar if wname == "wq" else nc.sync)
                    if first_w:
                        # tables ride the gpsimd SWDGE ring after the critical
                        # xT/wq transfers have been queued
                        nc.gpsimd.dma_start(out=cos_sb, in_=cos_d[:, :])
                        nc.gpsimd.dma_start(out=sin_sb, in_=sin_d[:, :])
                        nc.gpsimd.dma_start(out=et_sb, in_=et_d[:, :])
                        first_w = False
                    for mb in range(DT):
                        raw, raw_free = tc.tile([P, T], f16, name=f"raw{wname}_{mb}")
                        for nh in range(NJH):
                            pp = psx.tile([P, JH], f32, name="projps", tag="s",
                                          bufs=1)
                            for kb in range(KC):
                                nc.tensor.matmul(
                                    pp,
                                    w_sb[kb][:, mb * P:(mb + 1) * P],
                                    xT_sb[kb][:, nh * JH:(nh + 1) * JH],
                                    start=(kb == 0), stop=(kb == KC - 1),
                                )
                            if wname == "wq":
                                nc.scalar.copy(raw[:, nh * JH:(nh + 1) * JH], pp)
                            else:
                                nc.vector.tensor_copy(
                                    raw[:, nh * JH:(nh + 1) * JH], pp)
                        o = dest[mb]
                        nc.vector.stream_shuffle(rot_tmp, raw, shuf_mask)
                        nc.vector.tensor_mul(o, raw, cos_sb)
                        nc.vector.tensor_mul(rot_tmp, rot_tmp, sin_sb)
                        nc.vector.tensor_add(o, o, rot_tmp)
                        raw_free()
                        if wname == "wq":
                            emit_A(2 * mb, 0)
                            emit_A(2 * mb + 1, 0)
                        else:
                            emit_A(2 * mb, 1)
                            emit_A(2 * mb + 1, 1)
                    for fr in reversed(w_frees):
                        fr()

                # ---------------- v projection happens inside the steady
                # stream fill steps (t < 0) below ------------------------
                wv_sb, wv_frees = load_w(wv_d, "wv", nc.scalar)
                wo_sb = []
                for hc in range(DT):
                    t_ = persist.tile([P, C], f16, name=f"wo_{hc}",
                                      tag=f"wo{hc}")
                    nc.scalar.dma_start(out=t_, in_=wo_d[hc * P:(hc + 1) * P, :])
                    wo_sb.append(t_)

                def emit_v(tb):
                    pp = psx.tile([P, D], f32, name="vps", tag="av", bufs=1)
                    for kb in range(KC):
                        nc.tensor.matmul(
                            pp,
                            xT_sb[kb][:, tb * P:(tb + 1) * P],
                            wv_sb[kb],
                            start=(kb == 0), stop=(kb == KC - 1),
                        )
                    nc.scalar.copy(v_sb[tb], pp)

                # ---------------- steady attention pipeline ----------------
                # flattened (block, head) stream k = ib*HCL + h. At step t:
                # A(t+18) feeds the DRAM skew roundtrip ~10 steps ahead of
                # its SE; SE(t+8) runs a block ahead of PT(t+5) (3-step lag
                # hides the S->add->exp chain); AV(t+2) trails PT by 3; the
                # per-block norm / transpose / out-projection trail the last
                # AV of each block.
                NT = NB * HCL
                for t in range(-8, NT + 3):
                    if -8 <= t < 0:
                        emit_v(t + 8)
                    if t == 0:
                        for fr in reversed(wv_frees):
                            fr()
                        for fr in reversed(xT_free):
                            fr()
                    ka = t + 18
                    if 2 * HCL <= ka < NT:
                        iba, ha = divmod(ka, HCL)
                        emit_A(ha, iba)
                    ks = t + HCL
                    if 0 <= ks < NT:
                        ib, h = divmod(ks, HCL)
                        emit_SE(h, ib)
                    kp = t + 5
                    if 0 <= kp < NT:
                        ib, h = divmod(kp, HCL)
                        emit_PT(h, ib)
                    if 0 <= t < NT:
                        ib, h = divmod(t, HCL)
                        emit_AV(h, ib)
                        if h == HCL - 1:
                            emit_norm(ib)
                    if t >= HCL and t % HCL == 0:
                        emit_tr(t // HCL - 1)
                    if t >= HCL + 1 and t % HCL == 1:
                        emit_out(t // HCL - 1, 0)
                    if t >= HCL + 2 and t % HCL == 2:
                        emit_out(t // HCL - 1, 1)

    nc.compile()
    return nc


# partition p (within a head's 64) holds head-dim SIGMA[p]; pairs
# (d, d+32) land 16 apart inside a 32-partition quadrant.
SIGMA = np.concatenate([
    np.arange(0, 16), np.arange(32, 48),
    np.arange(16, 32), np.arange(48, 64),
])


def _host_tables(T, hs):
    inv_freq = 1.0 / (10000.0 ** (np.arange(0, hs, 2, dtype=np.float64) / hs))
    t = np.arange(T, dtype=np.float64)
    fr = np.outer(inv_freq, t)                     # [hs/2, T]
    cosT = np.empty((128, T), np.float32)
    sinS = np.empty((128, T), np.float32)
    for blk in range(128 // hs):
        for p in range(hs):
            d = SIGMA[p]
            row = blk * hs + p
            cosT[row] = np.cos(fr[d % 32]).astype(np.float32)
            s = np.sin(fr[d % 32]).astype(np.float32)
            sinS[row] = -s if d < 32 else s
    return cosT.astype(np.float16), sinS.astype(np.float16)


def make_et8(E, T, scale=8.0):
    et8 = np.zeros((128, 2 * T), np.float32)
    etp = (scale * E.T[SIGMA]).astype(np.float32)   # [64, 2T-1] permuted rows
    et8[:HS, :E.shape[0]] = etp
    et8[HS:2 * HS, :E.shape[0]] = etp
    return et8.astype(np.float16)


def perm_cols(w, D):
    """Permute per-head 64-column blocks of w [C, D] by SIGMA."""
    idx = (np.arange(D) // HS) * HS + SIGMA[np.arange(D) % HS]
    return np.ascontiguousarray(w[:, idx])


def get_nc(T=SEQ, C=EMB, D=HC * HS):
    key = (T, C, D)
    if key not in _cache:
        _cache[key] = _build_nc(T, C, D)
    return _cache[key]


def kernel(x, wq, wk, wv, wo, bo, rel_pos_emb):
    from concourse.bass_utils import run_bass_kernel_spmd

    x = np.asarray(x, dtype=np.float32)
    wq = np.asarray(wq, dtype=np.float32)
    wk = np.asarray(wk, dtype=np.float32)
    wv = np.asarray(wv, dtype=np.float32)
    wo = np.asarray(wo, dtype=np.float32)
    bo = np.asarray(bo, dtype=np.float32)
    E = np.asarray(rel_pos_emb, dtype=np.float32)

    T, C, D = SEQ, EMB, HC * HS
    nc = get_nc(T, C, D)

    cosT, sinS = _host_tables(T, HS)
    et8 = make_et8(E, T)

    in_maps = []
    for core in range(N_CORES):
        b, g = divmod(core, 2)
        sl = slice(g * D, (g + 1) * D)
        in_maps.append({
            "xT": np.ascontiguousarray(x[b].T).astype(np.float16),
            "wq": (perm_cols(wq[:, sl], D)
                   * np.float32(0.125)).astype(np.float16),
            "wk": perm_cols(wk[:, sl], D).astype(np.float16),
            "wv": np.ascontiguousarray(wv[:, sl]).astype(np.float16),
            "wo": np.ascontiguousarray(wo[sl, :]).astype(np.float16),
            "cosT": cosT,
            "sinS": sinS,
            "et8": et8,
        })

    res = run_bass_kernel_spmd(nc, in_maps, core_ids=list(range(N_CORES)))
    out = np.empty((BATCH, T, C), np.float32)
    for b in range(BATCH):
        out[b] = (res.results[2 * b]["y"].astype(np.float32)
                  + res.results[2 * b + 1]["y"].astype(np.float32) + bo)
    return out


# revision 55
# speedup vs baseline: 1.0291x; 1.0291x over previous
"""Trainium2 Bass kernel for nn_MultiHeadAttention_37623913513495.

Multi-head attention with rotary embeddings and a relative-position bias
(einsum('bhid,ijd->bhij', q, rel_pos[j-i+T-1])), sharded over 8 NeuronCores
as 4 batches x 2 head-groups (8 heads each). Host sums the two partial
outputs per batch and adds the bias.

Device-side structure (per core), v2 engine-balanced pipeline:
  - host ships x already transposed (xT [C, T]); q/k projections produce
    qT/kT in [d, T] layout, v in [T, d]; rotary on DVE; 1/sqrt(hs) folded
    into wq on the host, E shipped pre-multiplied by 8 to compensate
  - the relative-position "skew" rel[i,j] = A[i, j-i+c] is realized by
    writing RAW A windows to DRAM (f16, evacuated from PSUM by the gpsimd
    engine) and re-reading them with a strided diagonal access pattern
  - scores: S psum + rel added on DVE -> f16; single Exp on ACT with
    fused row-sums via accum_out
  - P is transposed by PE is_transpose matmuls (f16 PSUM out, cheap evac)
  - AV is computed in [i, d] orientation (N=64 matmuls); per-head
    normalization by 1/rowsum is a per-partition scalar multiply on the
    AV output; the 8-head attn block is re-transposed (4 PE transposes)
    into attnT for the output projection, which is pipelined per block
"""

import numpy as np

HS = 64           # head size
NH = 16           # total heads
SEQ = 1024        # sequence length
EMB = 1024        # embedding dim
BATCH = 4
N_CORES = 8
HC = NH // 2      # heads per core

_cache = {}


def _build_nc(T, C, D, reps=1):
    import concourse.bass as bass
    import concourse.bacc as bacc
    import concourse.mybir as mybir
    import concourse.tile as tile
    from concourse.masks import make_identity

    dt = mybir.dt
    f32, f16 = dt.float32, dt.float16
    AF = mybir.ActivationFunctionType
    ALU = mybir.AluOpType

    P = 128
    NB = T // P              # row blocks
    KC = C // P              # contraction chunks over C
    DT = D // P              # qT/kT partition tiles
    HPT = P // HS            # heads per qT tile (2)
    HCL = D // HS            # heads on this core
    JH = min(512, T)         # half width
    NJH = T // JH            # halves per row
    WF = T + P               # A-window width
    NCH = T // P             # j chunks for PT/AV
    ACH = (512, 512, WF - 1024)   # A psum chunk widths (bank-aligned)

    nc = bacc.Bacc(None, target_bir_lowering=False, debug=False)

    xT_d = nc.dram_tensor("xT", [C, T], f16, kind="ExternalInput")
    wq_d = nc.dram_tensor("wq", [C, D], f16, kind="ExternalInput")
    wk_d = nc.dram_tensor("wk", [C, D], f16, kind="ExternalInput")
    wv_d = nc.dram_tensor("wv", [C, D], f16, kind="ExternalInput")
    wo_d = nc.dram_tensor("wo", [D, C], f16, kind="ExternalInput")
    cos_d = nc.dram_tensor("cosT", [P, T], f16, kind="ExternalInput")
    sin_d = nc.dram_tensor("sinS", [P, T], f16, kind="ExternalInput")
    et_d = nc.dram_tensor("et8", [P, 2 * T], f16, kind="ExternalInput")
    y_d = nc.dram_tensor("y", [T, C], f16, kind="ExternalOutput")

    with tile.TileContext(nc) as tc:
        with (
            tc.tile_pool(name="const", bufs=1) as const,
            tc.tile_pool(name="persist", bufs=1) as persist,
            tc.tile_pool(name="asb", bufs=8) as asb_pool,
            tc.tile_pool(name="relsb", bufs=26) as rel_pool,
            tc.tile_pool(name="sssb", bufs=3) as ss_pool,
            tc.tile_pool(name="psb", bufs=11) as p_pool,
            tc.tile_pool(name="ptsb", bufs=3) as pt_pool,
            tc.tile_pool(name="attnsb", bufs=2) as attn_pool,
            tc.tile_pool(name="small", bufs=2) as small,
            tc.tile_pool(name="outsb", bufs=4) as out_pool,
            tc.tile_pool(name="psum", bufs=2, space="PSUM") as psx,
            tc.tile_pool(name="adram", bufs=32, space="DRAM") as adram,
        ):
            for _rep in range(reps):
                # ---------------- xT load (sync ring) ----------------
                xT_sb, xT_free = [], []
                for cb in range(KC):
                    t, fr = tc.tile([P, T], f16, name=f"xT_{cb}")
                    nc.sync.dma_start(out=t, in_=xT_d[cb * P:(cb + 1) * P, :])
                    xT_sb.append(t)
                    xT_free.append(fr)

                # ---------------- constants ----------------
                ident_b = const.tile([P, P], f16)
                make_identity(nc, ident_b)
                cos_sb = const.tile([P, T], f16)
                sin_sb = const.tile([P, T], f16)
                et_sb = const.tile([P, 2 * T], f16)

                def load_w(wd, name, eng):
                    tiles, frees = [], []
                    for kb in range(KC):
                        t, fr = tc.tile([P, D], f16, name=f"{name}_{kb}")
                        eng.dma_start(out=t, in_=wd[kb * P:(kb + 1) * P, :])
                        tiles.append(t)
                        frees.append(fr)
                    return tiles, frees

                # rotary pairs (d, d+32) sit 16 apart within a 32-partition
                # quadrant (host permutes weights/E/tables to match).
                shuf_mask = [(i + 16) % 32 for i in range(32)]

                qT_sb = [persist.tile([P, T], f16, name=f"qT_{mb}", tag=f"qT{mb}")
                         for mb in range(DT)]
                kT_sb = [persist.tile([P, T], f16, name=f"kT_{mb}", tag=f"kT{mb}")
                         for mb in range(DT)]
                v_sb = [persist.tile([P, D], f16, name=f"v_{tb}", tag=f"v{tb}")
                        for tb in range(NB)]
                attnT_sb = persist.tile([P, DT, T], f16, name="attnT", tag="attnT")

                rel_tiles = [dict() for _ in range(HCL)]
                p_tiles = [dict() for _ in range(HCL)]
                pt_tiles = [dict() for _ in range(HCL)]
                sums_blk = {}
                av_blk = {}
                attn_blk = {}

                def mode_add(k):
                    # every 4th stream step computes exp(S+rel) via a DVE add
                    # of raw rel instead of the exp(S)*exp(A) product -- this
                    # shifts work from the exp-bound ACT engine onto DVE.
                    return False

                def emit_A(h, ib):
                    """A window (head h, block ib) -> DRAM -> skew readback."""
                    par = (h % HPT) * HS
                    qtile = qT_sb[h // HPT]
                    i0 = ib * P
                    w0 = (T - P) - i0
                    a_ps = psx.tile([P, WF], f32, name="a_ps", tag="a",
                                    bufs=1)
                    off = 0
                    for cw in ACH:
                        nc.tensor.matmul(
                            a_ps[:, off:off + cw],
                            qtile[par:par + HS, i0:i0 + P],
                            et_sb[par:par + HS, w0 + off:w0 + off + cw],
                            start=True, stop=True)
                        off += cw
                    a_sb = asb_pool.tile([P, WF], f16, name="a_sb", tag="a_sb")
                    if mode_add(ib * HCL + h):
                        nc.vector.tensor_copy(a_sb, a_ps)
                    else:
                        nc.scalar.activation(a_sb, a_ps, AF.Exp)
                    a_dr = adram.tile([P, WF], f16, name="a_dr", tag="a_dr")
                    nc.sync.dma_start(out=a_dr, in_=a_sb)
                    rel = rel_pool.tile([P, T], f16, name="rel", tag="rel")
                    skew = bass.AP(
                        tensor=a_dr.tensor,
                        offset=a_dr.offset + (P - 1),
                        ap=[[WF - 1, P], [1, T]],
                    )
                    nc.sync.dma_start(out=rel, in_=skew)
                    rel_tiles[h][ib] = rel

                def emit_SE(h, ib):
                    """scores -> exp(S+rel) with fused rowsums (two forms)."""
                    par = (h % HPT) * HS
                    qtile, ktile = qT_sb[h // HPT], kT_sb[h // HPT]
                    i0 = ib * P
                    rel = rel_tiles[h].pop(ib)
                    s_ps = psx.tile([P, T], f32, name="s_ps", tag="s", bufs=1)
                    for jh in range(NJH):
                        sl = slice(jh * JH, (jh + 1) * JH)
                        nc.tensor.matmul(
                            s_ps[:, sl],
                            qtile[par:par + HS, i0:i0 + P],
                            ktile[par:par + HS, sl],
                            start=True, stop=True)
                    if ib not in sums_blk:
                        sums_blk[ib] = small.tile([P, HCL], f32, name="sums",
                                                  tag="sums", bufs=2)
                    p_sb = p_pool.tile([P, T], f16, name="p_sb", tag="p_sb")
                    if mode_add(ib * HCL + h):
                        ss_sb = ss_pool.tile([P, T], f16, name="ss_sb",
                                             tag="ss")
                        nc.vector.tensor_add(ss_sb, s_ps, rel)
                        nc.scalar.activation(
                            p_sb, ss_sb, AF.Exp,
                            accum_out=sums_blk[ib][:, h:h + 1])
                    else:
                        es_sb = ss_pool.tile([P, T], f16, name="es_sb",
                                             tag="ss")
                        nc.scalar.activation(es_sb, s_ps, AF.Exp)
                        nc.vector.scalar_tensor_tensor(
                            p_sb, es_sb, 1.0, rel,
                            ALU.mult, ALU.mult,
                            accum_out=sums_blk[ib][:, h:h + 1])
                    p_tiles[h][ib] = p_sb

                def emit_PT(h, ib):
                    """transpose P via PE is_transpose (f16 psum out)."""
                    p_sb = p_tiles[h].pop(ib)
                    ptp = psx.tile([P, NCH * P], f16, name="ptp", tag="pt",
                                   bufs=2)
                    for jc in range(NCH):
                        nc.tensor.transpose(
                            ptp[:, jc * P:(jc + 1) * P],
                            p_sb[:, jc * P:(jc + 1) * P], ident_b)
                    pt_sb = pt_pool.tile([P, NCH, P], f16, name="pt_sb",
                                         tag="pt_sb")
                    nc.vector.tensor_copy(
                        pt_sb.rearrange("p a b -> p (a b)"), ptp)
                    pt_tiles[h][ib] = pt_sb

                def emit_AV(h, ib):
                    """AV in [i, d] orientation into the shared block psum."""
                    pt_sb = pt_tiles[h].pop(ib)
                    if ib not in av_blk:
                        av_blk[ib] = psx.tile([P, D], f32, name="av_ps",
                                              tag="av", bufs=1)
                    av = av_blk[ib]
                    for jc in range(NCH):
                        nc.tensor.matmul(
                            av[:, h * HS:(h + 1) * HS],
                            pt_sb[:, jc, :],
                            v_sb[jc][:, h * HS:(h + 1) * HS],
                            start=(jc == 0), stop=(jc == NCH - 1))

                def emit_norm(ib):
                    """normalize the 8-head attn block by 1/rowsum."""
                    sums = sums_blk.pop(ib)
                    av = av_blk.pop(ib)
                    rec = small.tile([P, HCL], f32, name="rec", tag="rec",
                                     bufs=2)
                    nc.vector.reciprocal(rec, sums)
                    attn_sb = attn_pool.tile([P, D], f16, name="attn_sb",
                                             tag="attn")
                    for h in range(HCL):
                        nc.vector.tensor_scalar_mul(
                            attn_sb[:, h * HS:(h + 1) * HS],
                            av[:, h * HS:(h + 1) * HS],
                            rec[:, h:h + 1])
                    attn_blk[ib] = attn_sb

                def emit_tr(ib):
                    """re-transpose the attn block into attnT."""
                    attn_sb = attn_blk.pop(ib)
                    trp = psx.tile([P, DT * P], f16, name="trp", tag="pt",
                                   bufs=2)
                    for k in range(DT):
                        nc.tensor.transpose(
                            trp[:, k * P:(k + 1) * P],
                            attn_sb[:, k * P:(k + 1) * P], ident_b)
                    nc.scalar.copy(
                        attnT_sb[:, :, ib * P:(ib + 1) * P],
                        trp.rearrange("p (a b) -> p a b", a=DT))

                def emit_out(ib, ch):
                    """final projection for row-block ib (all heads done)."""
                    op = psx.tile([P, JH], f32, name="ops", tag="pt",
                                  bufs=2)
                    for hc in range(DT):
                        nc.tensor.matmul(
                            op,
                            attnT_sb[:, hc, ib * P:(ib + 1) * P],
                            wo_sb[hc][:, ch * JH:(ch + 1) * JH],
                            start=(hc == 0), stop=(hc == DT - 1))
                    o_sb = out_pool.tile([P, JH], f16, name="o_sb",
                                         tag="o_sb")
                    nc.vector.tensor_copy(o_sb, op)
                    nc.scalar.dma_start(
                        out=y_d[ib * P:(ib + 1) * P, ch * JH:(ch + 1) * JH],
                        in_=o_sb)

                # ---------------- q/k projections + rotary ----------------
                rot_tmp = p_pool.tile([P, T], f16, name="rot_tmp", tag="rot",
                                      bufs=1)
                first_w = True
                for (wd, wname, dest) in ((wq_d, "wq", qT_sb), (wk_d, "wk", kT_sb)):
                    w_sb, w_frees = load_w(
                        wd, wname, nc.scalar if wname == "wq" else nc.sync)
                    if first_w:
                        # tables ride the gpsimd SWDGE ring after the critical
                        # xT/wq transfers have been queued
                        nc.gpsimd.dma_start(out=cos_sb, in_=cos_d[:, :])
                        nc.gpsimd.dma_start(out=sin_sb, in_=sin_d[:, :])
                        nc.gpsimd.dma_start(out=et_sb, in_=et_d[:, :])
                        first_w = False
                    for mb in range(DT):
                        raw, raw_free = tc.tile([P, T], f16, name=f"raw{wname}_{mb}")
                        for nh in range(NJH):
                            pp = psx.tile([P, JH], f32, name="projps", tag="s",
                                          bufs=1)
                            for kb in range(KC):
                                nc.tensor.matmul(
                                    pp,
                                    w_sb[kb][:, mb * P:(mb + 1) * P],
                                    xT_sb[kb][:, nh * JH:(nh + 1) * JH],
                                    start=(kb == 0), stop=(kb == KC - 1),
                                )
                            if wname == "wq":
                                nc.scalar.copy(raw[:, nh * JH:(nh + 1) * JH], pp)
                            else:
                                nc.vector.tensor_copy(
                                    raw[:, nh * JH:(nh + 1) * JH], pp)
                        o = dest[mb]
                        nc.vector.stream_shuffle(rot_tmp, raw, shuf_mask)
                        nc.vector.tensor_mul(o, raw, cos_sb)
                        nc.vector.tensor_mul(rot_tmp, rot_tmp, sin_sb)
                        nc.vector.tensor_add(o, o, rot_tmp)
                        raw_free()
                        if wname == "wq":
                            emit_A(2 * mb, 0)
                            emit_A(2 * mb + 1, 0)
                        else:
                            emit_A(2 * mb, 1)
                            emit_A(2 * mb + 1, 1)
                    for fr in reversed(w_frees):
                        fr()

                # ---------------- v projection happens inside the steady
                # stream fill steps (t < 0) below ------------------------
                wv_sb, wv_frees = load_w(wv_d, "wv", nc.scalar)
                wo_sb = []
                for hc in range(DT):
                    t_ = persist.tile([P, C], f16, name=f"wo_{hc}",
                                      tag=f"wo{hc}")
                    nc.scalar.dma_start(out=t_, in_=wo_d[hc * P:(hc + 1) * P, :])
                    wo_sb.append(t_)

                def emit_v(tb):
                    pp = psx.tile([P, D], f32, name="vps", tag="av", bufs=1)
                    for kb in range(KC):
                        nc.tensor.matmul(
                            pp,
                            xT_sb[kb][:, tb * P:(tb + 1) * P],
                            wv_sb[kb],
                            start=(kb == 0), stop=(kb == KC - 1),
                        )
                    nc.scalar.copy(v_sb[tb], pp)

                # ---------------- steady attention pipeline ----------------
                # flattened (block, head) stream k = ib*HCL + h. At step t:
                # A(t+18) feeds the DRAM skew roundtrip ~10 steps ahead of
                # its SE; SE(t+8) runs a block ahead of PT(t+5) (3-step lag
                # hides the S->add->exp chain); AV(t+2) trails PT by 3; the
                # per-block norm / transpose / out-projection trail the last
                # AV of each block.
                NT = NB * HCL
                for t in range(-8, NT + 3):
                    if -8 <= t < 0:
                        emit_v(t + 8)
                    if t == 0:
                        for fr in reversed(wv_frees):
                            fr()
                        for fr in reversed(xT_free):
                            fr()
                    ka = t + 18
                    if 2 * HCL <= ka < NT:
                        iba, ha = divmod(ka, HCL)
                        emit_A(ha, iba)
                    ks = t + HCL
                    if 0 <= ks < NT:
                        ib, h = divmod(ks, HCL)
                        emit_SE(h, ib)
                    kp = t + 4
                    if 0 <= kp < NT:
                        ib, h = divmod(kp, HCL)
                        emit_PT(h, ib)
                    if 0 <= t < NT:
                        ib, h = divmod(t, HCL)
                        emit_AV(h, ib)
                        if h == HCL - 1:
                            emit_norm(ib)
                    if t >= HCL and t % HCL == 0:
                        emit_tr(t // HCL - 1)
                    if t >= HCL + 1 and t % HCL == 1:
                        emit_out(t // HCL - 1, 0)
                    if t >= HCL + 2 and t % HCL == 2:
                        emit_out(t // HCL - 1, 1)

    nc.compile()
    return nc


# partition p (within a head's 64) holds head-dim SIGMA[p]; pairs
# (d, d+32) land 16 apart inside a 32-partition quadrant.
SIGMA = np.concatenate([
    np.arange(0, 16), np.arange(32, 48),
    np.arange(16, 32), np.arange(48, 64),
])


def _host_tables(T, hs):
    inv_freq = 1.0 / (10000.0 ** (np.arange(0, hs, 2, dtype=np.float64) / hs))
    t = np.arange(T, dtype=np.float64)
    fr = np.outer(inv_freq, t)                     # [hs/2, T]
    cosT = np.empty((128, T), np.float32)
    sinS = np.empty((128, T), np.float32)
    for blk in range(128 // hs):
        for p in range(hs):
            d = SIGMA[p]
            row = blk * hs + p
            cosT[row] = np.cos(fr[d % 32]).astype(np.float32)
            s = np.sin(fr[d % 32]).astype(np.float32)
            sinS[row] = -s if d < 32 else s
    return cosT.astype(np.float16), sinS.astype(np.float16)


def make_et8(E, T, scale=8.0):
    et8 = np.zeros((128, 2 * T), np.float32)
    etp = (scale * E.T[SIGMA]).astype(np.float32)   # [64, 2T-1] permuted rows
    et8[:HS, :E.shape[0]] = etp
    et8[HS:2 * HS, :E.shape[0]] = etp
    return et8.astype(np.float16)


def perm_cols(w, D):
    """Permute per-head 64-column blocks of w [C, D] by SIGMA."""
    idx = (np.arange(D) // HS) * HS + SIGMA[np.arange(D) % HS]
    return np.ascontiguousarray(w[:, idx])


def get_nc(T=SEQ, C=EMB, D=HC * HS):
    key = (T, C, D)
    if key not in _cache:
        _cache[key] = _build_nc(T, C, D)
    return _cache[key]


def kernel(x, wq, wk, wv, wo, bo, rel_pos_emb):
    from concourse.bass_utils import run_bass_kernel_spmd

    x = np.asarray(x, dtype=np.float32)
    wq = np.asarray(wq, dtype=np.float32)
    wk = np.asarray(wk, dtype=np.float32)
    wv = np.asarray(wv, dtype=np.float32)
    wo = np.asarray(wo, dtype=np.float32)
    bo = np.asarray(bo, dtype=np.float32)
    E = np.asarray(rel_pos_emb, dtype=np.float32)

    T, C, D = SEQ, EMB, HC * HS
    nc = get_nc(T, C, D)

    cosT, sinS = _host_tables(T, HS)
    et8 = make_et8(E, T)

    in_maps = []
    for core in range(N_CORES):
        b, g = divmod(core, 2)
        sl = slice(g * D, (g + 1) * D)
        in_maps.append({
            "xT": np.ascontiguousarray(x[b].T).astype(np.float16),
            "wq": (perm_cols(wq[:, sl], D)
                   * np.float32(0.125)).astype(np.float16),
            "wk": perm_cols(wk[:, sl], D).astype(np.float16),
            "wv": np.ascontiguousarray(wv[:, sl]).astype(np.float16),
            "wo": np.ascontiguousarray(wo[sl, :]).astype(np.float16),
            "cosT": cosT,
            "sinS": sinS,
            "et8": et8,
        })

    res = run_bass_kernel_spmd(nc, in_maps, core_ids=list(range(N_CORES)))
    out = np.empty((BATCH, T, C), np.float32)
    for b in range(BATCH):
        out[b] = (res.results[2 * b]["y"].astype(np.float32)
                  + res.results[2 * b + 1]["y"].astype(np.float32) + bo)
    return out
